# revision 9
# baseline (speedup 1.0000x reference)
"""Trainium2 Bass kernel for nn_AttenPool_22917945491863.

Mathematical reduction: in the reference, ``attn`` is softmaxed over axis 3
and then summed over that same axis — the sum of a softmax over its own axis
is exactly 1, so the whole query branch (2 convs, BN, ReLU, LayerNorm,
softmax) collapses to ``a = ones``. The remaining computation

    out = sumpool4x4((1-alpha) * (conv3x3(bn(x), wv) + bv) + alpha * x)

is a 6x6 stride-4 convolution over zero-padded x (sumpool of a 3x3 conv is a
6x6 stride-4 conv with summed taps; the BN scale folds into the weights; the
BN shift and conv bias fold into a precomputed per-output-position bias map;
the alpha*x sum-pool folds in as a depthwise component on the central 4x4
taps).

Device mapping (8 cores, batch-parallel, 2 samples each):
  - x ships as fp8 e3m4 (1 byte/elem) with host-side 2D Floyd-Steinberg
    error diffusion: the quantization noise lands at high spatial
    frequency, which cancels in the 4x4 sum-pool and the slowly-varying
    conv weights (rel err ~6.7e-3 vs 1.25e-2 plain e3m4). Weights stay
    fp16 (mixed-dtype matmul is legal for non-fp32 operands).
  - Zero-padded h-parity, phase-major column layout: partition p holds
    channel (p % 64); p<64 even padded rows, p>=64 odd; padded col c at
    (c%4)*33 + c//4 so each tap's 32 stride-4 columns are contiguous.
    K=128 contracts 64 channels x 2 vertically-adjacent taps.
  - 36 taps -> 9 pair-steps per output tile, each a column-tiled pair of
    [K=128, M=64, N=256] matmuls at tile_position (0,0)/(0,64); 8 tiles
    of 8 ph-rows pipelined against 8 x-chunk DMAs on the Sync ring.
    Weights are step-major; the first 3 steps (w_a) gate the PE so it
    starts ~1.5us before the rest of w lands.
  - ACT ring: w_a, w_b, abias (all fp16), then fp16 outputs (one DMA for
    sample 0, sample 1 split so the final piece has a small descgen).
  - The PE HAM clock gate needs one FULL free-running 4096-cycle window
    (3.4us) of busy to lift 1.2 -> 2.4 GHz, so ~36 dummy matmuls on
    garbage SBUF keep the PE busy from block entry straight into the
    real stream (any idle gap restarts the window). They target tile
    7's PSUM bank, which the real tile-7 group's start=True clears.
  - The bass-init const memsets + barrier are patched out (~0.45us off
    the critical path; nothing in this kernel uses the const APs).
"""

import numpy as np

B, C, H, W = 16, 64, 128, 128
NCORES = 8
BPC = B // NCORES  # samples per core
OH = OW = 32  # output spatial
WPAD = 132  # padded row length: stored phase-major as [4 phases][33 cols]
NROW = 65  # padded rows per parity block
EPS = 1e-5
HOST_CHUNKS = ((0, 18), (18, 34), (34, 50), (50, NROW))
N_WARMUP_MM = 36
WA_STEPS = 3  # steps covered by the first weight DMA

_PROGRAM_CACHE = {}


def _build_program():
    import concourse.bacc as bacc
    import concourse.bass as bass
    import concourse.mybir as mybir

    class _NoBarrierBlock(bass.BassBlock):
        """BassBlock whose exit drains each used engine but skips the
        all-engine EVSEM butterfly barrier: the NRT postamble's own
        barrier + semaphore wipe and per-engine DGE drains already
        guarantee completion, so the extra barrier only adds latency."""

        def __exit__(self, exc_type, exc_val, exc_tb):
            if exc_type is not None:
                return
            for engine, last_body in self.last_body.items():
                with self.bass.body(last_body, parent=self.bass.cur_bb,
                                    allow_existing_parent=True):
                    engine.br(self.end_bb)
            self.bass.switch_bb(self.end_bb)
            gpsimd_type = self.bass.gpsimd.engine
            for eng_type, eng in self.bass.engines.items():
                if eng_type == gpsimd_type:
                    continue
                d = mybir.InstDrain(
                    name=self.bass.get_next_instruction_name(),
                    ins=[], outs=[], bass_is_fusable=False)
                d.engine = eng_type
                eng.add_instruction(d)

    f32 = mybir.dt.float32
    f16 = mybir.dt.float16
    xdt = mybir.dt.float8e3  # e3m4: 4 mantissa bits, best fp8 for N(0,1) x

    # Bass.__init__ unconditionally emits 4 const-AP memsets + an
    # all-engine barrier (~0.45us) ahead of the first real instruction.
    # This kernel never touches the const APs, so patch them out for the
    # duration of program construction.
    _skip = lambda self, *a, **k: None
    patches = [(bass.Bass, "all_engine_barrier"),
               (bass.BassSharedVectorInterface, "memset"),
               (bass.BassEitherVectorEngine, "memset")]
    saved = [(cls, name, getattr(cls, name)) for cls, name in patches]
    for cls, name in patches:
        setattr(cls, name, _skip)
    try:
        nc = bacc.Bacc("TRN2", target_bir_lowering=False, debug=False,
                       num_devices=NCORES)
    finally:
        for cls, name, fn in saved:
            setattr(cls, name, fn)

    # x is stored chunk-major on the host (each chunk's [128, rows*132]
    # block flattened partition-major) so every chunk DMA reads one fully
    # contiguous DRAM region
    xp = nc.dram_tensor("xp", [BPC, 128 * NROW * WPAD], xdt,
                        kind="ExternalInput").ap()
    w_in = nc.dram_tensor("w", [128, 18 * 64], f16, kind="ExternalInput").ap()
    ab_in = nc.dram_tensor("abias", [C, OH * OW], f16,
                           kind="ExternalInput").ap()
    out = nc.dram_tensor("out", [BPC, C, OH * OW], f16,
                         kind="ExternalOutput").ap()

    x2 = [nc.alloc_sbuf_tensor(f"x2_{b}", [128, NROW * WPAD], xdt).ap()
          for b in range(BPC)]
    w_sb = nc.alloc_sbuf_tensor("w_sb", [128, 18 * 64], f16).ap()
    ab_sb = nc.alloc_sbuf_tensor("ab_sb", [C, OH * OW], f16).ap()
    # per-sample output buffer; tiles write disjoint 256-col ranges so one
    # DMA per sample drains all 4 tiles
    ob = [nc.alloc_sbuf_tensor(f"ob_{b}", [C, OH * OW], f16).ap()
          for b in range(BPC)]
    tmp = nc.alloc_sbuf_tensor("tmp", [C, 32 * 8], f32).ap()
    NCH = len(HOST_CHUNKS)
    TILES = [(b, 8 * j, 8, NCH * b + j) for b in range(BPC) for j in range(4)]
    # [128, N] PSUM per tile: partitions 0-63 accumulate the step's first
    # pair (column groups 0-1 of the PE array), partitions 64-127 the
    # second pair (groups 2-3); DVE folds the halves in the epilogue.
    ps = [nc.alloc_psum_tensor(f"ps_{t}", [128, 32 * nph], f32).ap()
          for t, (_, _, nph, _) in enumerate(TILES)]

    WACOL = 128 * WA_STEPS
    wsem_a = nc.alloc_semaphore("wsem_a")  # w steps 0-2 landed
    wsem_b = nc.alloc_semaphore("wsem_b")  # w steps 3-8 landed
    absem = nc.alloc_semaphore("absem")    # abias landed
    csem = [nc.alloc_semaphore(f"csem{i}") for i in range(NCH * BPC)]
    mmsem = nc.alloc_semaphore("mmsem")  # 2 incs per tile (per col group)
    vsem = nc.alloc_semaphore("vsem")    # per-tile bias add done
    osem = nc.alloc_semaphore("osem")    # output DMAs landed (never waited)

    with _NoBarrierBlock(nc, "main") as block:

        @block.sync
        def _(sync):
            # Sync ring: the 8 x chunks in consumption order; back-to-back
            # queued transfers keep the SDMA engines at stream rate
            ci = 0
            for b in range(BPC):
                off = 0
                for r0, r1 in HOST_CHUNKS:
                    n = (r1 - r0) * WPAD
                    src = xp[b, off * 128:(off + n) * 128].rearrange(
                        "(p n) -> p n", n=n)
                    sync.dma_start(
                        out=x2[b][:, r0 * WPAD:r1 * WPAD], in_=src,
                    ).then_inc(csem[ci], 16)
                    ci += 1
                    off += n

        @block.scalar
        def _(scalar):
            # ACT ring runs in parallel with the x stream: the PE gate
            # (w_a) first, then the rest of w, then abias (DVE needs it
            # ~3us in), then the outputs
            scalar.dma_start(out=w_sb[:, 0:WACOL],
                             in_=w_in[:, 0:WACOL]).then_inc(wsem_a, 16)
            scalar.dma_start(out=w_sb[:, WACOL:],
                             in_=w_in[:, WACOL:]).then_inc(wsem_b, 16)
            scalar.dma_start(out=ab_sb[:], in_=ab_in[:]).then_inc(absem, 16)
            scalar.wait_ge(vsem, 4)
            scalar.dma_start(out=out[0], in_=ob[0][:]).then_inc(osem, 16)
            # sample 1 in two pieces so the final, critical-path DMA has a
            # small descriptor-generation cost
            scalar.wait_ge(vsem, 7)
            scalar.dma_start(out=out[1, :, 0:768],
                             in_=ob[1][:, 0:768]).then_inc(osem, 16)
            scalar.wait_ge(vsem, 8)
            scalar.dma_start(out=out[1, :, 768:1024],
                             in_=ob[1][:, 768:1024]).then_inc(osem, 16)
            # no final osem wait: the NRT postamble's per-engine DGE
            # drains guarantee the last write completes before NEFF end

        @block.tensor
        def _(tensor):
            # HAM warmup on garbage SBUF: PE must be busy for one FULL
            # free-running 3.4us activity window to clock up, and any idle
            # gap restarts the window — so bridge block entry -> first
            # real matmul (~3.8us) without pause.
            for i in range(N_WARMUP_MM):
                tensor.matmul(ps[7][0:64, 0:128], w_sb[:, 0:64],
                              w_sb[:, 256:384], start=True, stop=True,
                              tile_position=(0, 0))
            tensor.wait_ge(wsem_a, 16)
            for t, (b, p0, nph, ci) in enumerate(TILES):
                tensor.wait_ge(csem[ci], 16)
                if t == 0:
                    wb_wait = WA_STEPS  # wait wsem_b before this step
                else:
                    wb_wait = -1
                v = x2[b].rearrange("p (r f c) -> p r f c", f=4, c=33)
                # column-tiled pairs: step i runs pair i in PE columns
                # 0-63 and pair 9+i concurrently in columns 64-127
                for i in range(9):
                    if i == wb_wait:
                        tensor.wait_ge(wsem_b, 16)
                    for g in range(2):
                        j = 9 * g + i
                        a, sw = divmod(j, 6)
                        r0 = 2 * p0 + a
                        rhs = v[:, r0: r0 + 2 * nph - 1: 2, sw % 4,
                                sw // 4: sw // 4 + 32]
                        mm = tensor.matmul(
                            ps[t][64 * g:64 * g + 64, :],
                            w_sb[:, 128 * i + 64 * g:128 * i + 64 * g + 64],
                            rhs, start=(i == 0), stop=(i == 8),
                            tile_position=(0, 64 * g))
                        if i == 8:
                            # two incs per tile: DVE starts folding the
                            # finished half while the other drains
                            mm.then_inc(mmsem, 1)

        @block.vector
        def _(vector):
            vector.wait_ge(absem, 16)
            for t, (b, p0, nph, _) in enumerate(TILES):
                # group 0 (psum partitions 0-63) retires first: fold it
                # with the bias while group 1's last matmul drains.
                # DVE reads at most one PSUM operand per op.
                vector.wait_ge(mmsem, 2 * t + 1)
                vector.tensor_add(tmp[:], ps[t][0:64, :],
                                  ab_sb[:, p0 * 32:(p0 + nph) * 32])
                vector.wait_ge(mmsem, 2 * t + 2)
                vector.tensor_add(ob[b][:, p0 * 32:(p0 + nph) * 32],
                                  tmp[:], ps[t][64:128, :]).then_inc(vsem, 1)

    nc.compile()
    return nc


def _host_precompute(inputs):
    """Fold BN/alpha/bias into 6x6 stride-4 conv weights + bias map (f64)."""
    g0 = np.asarray(inputs["g0"], np.float64)
    b0 = np.asarray(inputs["b0"], np.float64)
    m0 = np.asarray(inputs["m0"], np.float64)
    v0 = np.asarray(inputs["v0"], np.float64)
    wv = np.asarray(inputs["wv"], np.float64)
    bv = np.asarray(inputs["bv"], np.float64)
    alpha = float(np.asarray(inputs["alpha"]))

    s0 = g0 / np.sqrt(v0 + EPS)
    t0 = b0 - m0 * s0

    # W'[o,c,sh,sw] = sum of 3x3 taps t with s - t in [0,4)^2
    Wp = np.zeros((C, C, 6, 6))
    for sh in range(6):
        for sw in range(6):
            th0, th1 = max(0, sh - 3), min(3, sh + 1)
            tw0, tw1 = max(0, sw - 3), min(3, sw + 1)
            Wp[:, :, sh, sw] = wv[:, :, th0:th1, tw0:tw1].sum(axis=(2, 3))

    W_final = (1.0 - alpha) * Wp * s0[None, :, None, None]
    idx = np.arange(C)
    for sh in range(1, 5):
        for sw in range(1, 5):
            W_final[idx, idx, sh, sw] += alpha

    # bias map: contribution of the BN shift t0 through the conv (with
    # zero-padding mask) plus conv bias, scaled by (1-alpha)
    Rm = np.zeros((OH, 6))
    for p in range(OH):
        for s in range(6):
            if 0 <= 4 * p + s - 1 < H:
                Rm[p, s] = 1.0
    A0 = np.einsum("ocuv,pu,qv,c->opq", Wp, Rm, Rm, t0)
    Abias = (1.0 - alpha) * (A0 + 16.0 * bv[:, None, None])

    # step-major lhsT layout: step i holds pair i (cols 0-63) then pair
    # 9+i (cols 64-127); pair j = (a, sw) = divmod(j, 6), rows 0-63 =
    # tap (2a, sw), rows 64-127 = tap (2a+1, sw); [k, m] = [ci, co]
    W18 = np.zeros((128, 18 * 64))
    for i in range(9):
        for g in range(2):
            j = 9 * g + i
            a, sw = divmod(j, 6)
            c0 = 128 * i + 64 * g
            W18[0:64, c0:c0 + 64] = W_final[:, :, 2 * a, sw].T
            W18[64:128, c0:c0 + 64] = W_final[:, :, 2 * a + 1, sw].T

    return W18, Abias.reshape(C, OH * OW)


def _fs_dither_e3m4(x):
    """Quantize x to fp8 e3m4 with 2D Floyd-Steinberg error diffusion.

    The output sums x over 4x4 patches (the alpha residual) and contracts
    it against slowly-varying 6x6 conv weights, so pushing the quantization
    error to high spatial frequency roughly halves the end-to-end error vs
    plain round-to-nearest."""
    import ml_dtypes

    v = np.asarray(x, np.float32).reshape(-1, H, W).copy()
    out = np.empty_like(v)
    sev = np.float32(7 / 16)
    dl, dc, dr = np.float32(3 / 16), np.float32(5 / 16), np.float32(1 / 16)
    for h in range(H):
        nxt = np.zeros((v.shape[0], W), np.float32)
        row = v[:, h]
        for w in range(W):
            val = row[:, w]
            qv = val.astype(ml_dtypes.float8_e3m4)
            out[:, h, w] = qv
            e = val - qv.astype(np.float32)
            if w + 1 < W:
                row[:, w + 1] += e * sev
                nxt[:, w + 1] += e * dr
            nxt[:, w] += e * dc
            if w > 0:
                nxt[:, w - 1] += e * dl
        if h + 1 < H:
            v[:, h + 1] += nxt
    return out.reshape(x.shape).astype(ml_dtypes.float8_e3m4)


def _host_shuffle_x(x):
    """Zero-padded h-parity, phase-major-column fp8 layout
    [B, 128*NROW*WPAD], chunk-major.

    Partition p < 64: channel p, even padded rows (pad row 2*r -> h=2r-1);
    partition p >= 64: channel p-64, odd padded rows (pad row 2*r+1 -> h=2r).
    Padded col c (data cols 1..128, zeros at 0/129/130/131) is stored at
    row offset (c%4)*33 + c//4 so stride-4 tap reads are contiguous.
    """
    import ml_dtypes

    xq = _fs_dither_e3m4(x)
    xpad = np.zeros((B, 128, NROW, WPAD), ml_dtypes.float8_e3m4)
    xpad[:, 0:64, 1:65, 1:129] = xq[:, :, 1::2, :]
    xpad[:, 64:128, 0:64, 1:129] = xq[:, :, 0::2, :]
    # c = cc*4 + phase -> phase-major [4][33]
    xph = np.ascontiguousarray(
        xpad.reshape(B, 128, NROW, 33, 4).transpose(0, 1, 2, 4, 3)
    ).reshape(B, 128, NROW, WPAD)
    # chunk-major: concatenate each row-chunk's [128, rows*WPAD] block so
    # the device reads one contiguous DRAM region per chunk DMA
    blocks = []
    for r0, r1 in HOST_CHUNKS:
        blocks.append(xph[:, :, r0:r1, :].reshape(B, 128 * (r1 - r0) * WPAD))
    return np.ascontiguousarray(np.concatenate(blocks, axis=1))


def _prepare_in_maps(inputs):
    x = np.asarray(inputs["x"], np.float32)
    W18, Abias = _host_precompute(inputs)
    w_host = np.ascontiguousarray(W18.astype(np.float16))
    ab_host = np.ascontiguousarray(Abias.astype(np.float16))
    xp = _host_shuffle_x(x)
    return [
        {"xp": xp[i * BPC:(i + 1) * BPC], "w": w_host, "abias": ab_host}
        for i in range(NCORES)
    ]


def kernel(**inputs):
    from concourse.bass_utils import run_bass_kernel_spmd

    in_maps = _prepare_in_maps(inputs)

    if "nc" not in _PROGRAM_CACHE:
        _PROGRAM_CACHE["nc"] = _build_program()
    nc = _PROGRAM_CACHE["nc"]

    res = run_bass_kernel_spmd(nc, in_maps, list(range(NCORES)))
    out = np.concatenate(
        [np.asarray(res.results[i]["out"]).astype(np.float32).reshape(
            BPC, C, OH, OW) for i in range(NCORES)],
        axis=0,
    )
    return np.ascontiguousarray(out)


# revision 14
# speedup vs baseline: 1.2749x; 1.2749x over previous
"""Trainium2 Bass kernel for nn_AttenPool_22917945491863.

Mathematical reduction: in the reference, ``attn`` is softmaxed over axis 3
and then summed over that same axis — the sum of a softmax over its own axis
is exactly 1, so the whole query branch (2 convs, BN, ReLU, LayerNorm,
softmax) collapses to ``a = ones``. The remaining computation

    out = sumpool4x4((1-alpha) * (conv3x3(bn(x), wv) + bv) + alpha * x)

is a 6x6 stride-4 convolution over zero-padded x (sumpool of a 3x3 conv is a
6x6 stride-4 conv with summed taps; the BN scale folds into the weights; the
BN shift and conv bias fold into a precomputed per-output-position bias map;
the alpha*x sum-pool folds in as a depthwise component on the central 4x4
taps).

Device mapping (8 cores, batch-parallel, 2 samples each):
  - x ships as fp8 e3m4 (1 byte/elem) with host-side 2D Floyd-Steinberg
    error diffusion: the quantization noise lands at high spatial
    frequency, which cancels in the 4x4 sum-pool and the slowly-varying
    conv weights (rel err ~6.7e-3 vs 1.25e-2 plain e3m4). Weights stay
    fp16 (mixed-dtype matmul is legal for non-fp32 operands).
  - Zero-padded h-parity, phase-major column layout: partition p holds
    channel (p % 64); p<64 even padded rows, p>=64 odd; padded col c at
    (c%4)*33 + c//4 so each tap's 32 stride-4 columns are contiguous.
    K=128 contracts 64 channels x 2 vertically-adjacent taps.
  - 36 taps -> 9 pair-steps per output tile, each a column-tiled pair of
    [K=128, M=64, N=256] matmuls at tile_position (0,0)/(0,64); 8 tiles
    of 8 ph-rows pipelined against 8 x-chunk DMAs on the Sync ring.
    Weights are step-major; the first 3 steps (w_a) gate the PE so it
    starts ~1.5us before the rest of w lands.
  - ACT ring: w_a, w_b, abias (all fp16), then fp16 outputs (one DMA for
    sample 0, sample 1 split so the final piece has a small descgen).
  - The PE HAM clock gate needs one FULL free-running 4096-cycle window
    (3.4us) of busy to lift 1.2 -> 2.4 GHz, so ~36 dummy matmuls on
    garbage SBUF keep the PE busy from block entry straight into the
    real stream (any idle gap restarts the window). They target tile
    7's PSUM bank, which the real tile-7 group's start=True clears.
  - The bass-init const memsets + barrier are patched out (~0.45us off
    the critical path; nothing in this kernel uses the const APs).
"""

import numpy as np

B, C, H, W = 16, 64, 128, 128
NCORES = 8
BPC = B // NCORES  # samples per core
OH = OW = 32  # output spatial
WPAD = 132  # padded row length: stored phase-major as [4 phases][33 cols]
NROW = 65  # padded rows per parity block
EPS = 1e-5
HOST_CHUNKS = ((0, 18), (18, 34), (34, 50), (50, NROW))
N_WARMUP_MM = 28
WA_STEPS = 3  # steps covered by the first weight DMA

_PROGRAM_CACHE = {}


def _build_program():
    import concourse.bacc as bacc
    import concourse.bass as bass
    import concourse.mybir as mybir

    class _NoBarrierBlock(bass.BassBlock):
        """BassBlock whose exit drains each used engine but skips the
        all-engine EVSEM butterfly barrier: the NRT postamble's own
        barrier + semaphore wipe and per-engine DGE drains already
        guarantee completion, so the extra barrier only adds latency."""

        def __exit__(self, exc_type, exc_val, exc_tb):
            if exc_type is not None:
                return
            for engine, last_body in self.last_body.items():
                with self.bass.body(last_body, parent=self.bass.cur_bb,
                                    allow_existing_parent=True):
                    engine.br(self.end_bb)
            self.bass.switch_bb(self.end_bb)
            gpsimd_type = self.bass.gpsimd.engine
            for eng_type, eng in self.bass.engines.items():
                if eng_type == gpsimd_type:
                    continue
                d = mybir.InstDrain(
                    name=self.bass.get_next_instruction_name(),
                    ins=[], outs=[], bass_is_fusable=False)
                d.engine = eng_type
                eng.add_instruction(d)

    f32 = mybir.dt.float32
    f16 = mybir.dt.float16
    xdt = mybir.dt.float8e3  # e3m4: 4 mantissa bits, best fp8 for N(0,1) x

    # Bass.__init__ unconditionally emits 4 const-AP memsets + an
    # all-engine barrier (~0.45us) ahead of the first real instruction.
    # This kernel never touches the const APs, so patch them out for the
    # duration of program construction.
    _skip = lambda self, *a, **k: None
    patches = [(bass.Bass, "all_engine_barrier"),
               (bass.BassSharedVectorInterface, "memset"),
               (bass.BassEitherVectorEngine, "memset")]
    saved = [(cls, name, getattr(cls, name)) for cls, name in patches]
    for cls, name in patches:
        setattr(cls, name, _skip)
    try:
        nc = bacc.Bacc("TRN2", target_bir_lowering=False, debug=False,
                       num_devices=NCORES)
    finally:
        for cls, name, fn in saved:
            setattr(cls, name, fn)

    # x is stored chunk-major on the host (each chunk's [128, rows*132]
    # block flattened partition-major) so every chunk DMA reads one fully
    # contiguous DRAM region
    xp = nc.dram_tensor("xp", [BPC, 128 * NROW * WPAD], xdt,
                        kind="ExternalInput").ap()
    # two separate (contiguous) weight tensors: a DRAM-side slice of one
    # tensor would make 128 strided sub-KB reads, which run at ~90 GB/s
    # and throttle the whole SDMA fabric
    WACOL = 128 * WA_STEPS
    wa_in = nc.dram_tensor("wa", [128, WACOL], f16,
                           kind="ExternalInput").ap()
    wb_in = nc.dram_tensor("wb", [128, 18 * 64 - WACOL], f16,
                           kind="ExternalInput").ap()
    ab_in = nc.dram_tensor("abias", [C, OH * OW], f16,
                           kind="ExternalInput").ap()
    out = nc.dram_tensor("out", [BPC, C, OH * OW], f16,
                         kind="ExternalOutput").ap()

    x2 = [nc.alloc_sbuf_tensor(f"x2_{b}", [128, NROW * WPAD], xdt).ap()
          for b in range(BPC)]
    w_sb = nc.alloc_sbuf_tensor("w_sb", [128, 18 * 64], f16).ap()
    ab_sb = nc.alloc_sbuf_tensor("ab_sb", [C, OH * OW], f16).ap()
    # per-sample output buffer; tiles write disjoint 256-col ranges so one
    # DMA per sample drains all 4 tiles
    ob = [nc.alloc_sbuf_tensor(f"ob_{b}", [C, OH * OW], f16).ap()
          for b in range(BPC)]
    tmp = nc.alloc_sbuf_tensor("tmp", [C, 32 * 8], f32).ap()
    NCH = len(HOST_CHUNKS)
    TILES = [(b, 8 * j, 8, NCH * b + j) for b in range(BPC) for j in range(4)]
    # [128, N] PSUM per tile: partitions 0-63 accumulate the step's first
    # pair (column groups 0-1 of the PE array), partitions 64-127 the
    # second pair (groups 2-3); DVE folds the halves in the epilogue.
    ps = [nc.alloc_psum_tensor(f"ps_{t}", [128, 32 * nph], f32).ap()
          for t, (_, _, nph, _) in enumerate(TILES)]

    wsem_a = nc.alloc_semaphore("wsem_a")  # w steps 0-2 landed
    wsem_b = nc.alloc_semaphore("wsem_b")  # w steps 3-8 landed
    absem = nc.alloc_semaphore("absem")    # abias landed
    csem = [nc.alloc_semaphore(f"csem{i}") for i in range(NCH * BPC)]
    mmsem = nc.alloc_semaphore("mmsem")  # 2 incs per tile (per col group)
    vsem = nc.alloc_semaphore("vsem")    # per-tile bias add done
    osem = nc.alloc_semaphore("osem")    # output DMAs landed (never waited)

    with _NoBarrierBlock(nc, "main") as block:

        @block.sync
        def _(sync):
            # Sync ring: the 8 x chunks in consumption order; back-to-back
            # queued transfers keep the SDMA engines at stream rate
            ci = 0
            for b in range(BPC):
                off = 0
                for r0, r1 in HOST_CHUNKS:
                    n = (r1 - r0) * WPAD
                    src = xp[b, off * 128:(off + n) * 128].rearrange(
                        "(p n) -> p n", n=n)
                    sync.dma_start(
                        out=x2[b][:, r0 * WPAD:r1 * WPAD], in_=src,
                    ).then_inc(csem[ci], 16)
                    ci += 1
                    off += n

        @block.scalar
        def _(scalar):
            # ACT ring runs in parallel with the x stream: the PE gate
            # (w_a) first, then the rest of w, then abias (DVE needs it
            # ~3us in), then the outputs
            scalar.dma_start(out=w_sb[:, 0:WACOL],
                             in_=wa_in[:]).then_inc(wsem_a, 16)
            scalar.dma_start(out=w_sb[:, WACOL:],
                             in_=wb_in[:]).then_inc(wsem_b, 16)
            scalar.dma_start(out=ab_sb[:], in_=ab_in[:]).then_inc(absem, 16)
            scalar.wait_ge(vsem, 4)
            scalar.dma_start(out=out[0], in_=ob[0][:]).then_inc(osem, 16)
            # sample 1 in two pieces so the final, critical-path DMA has a
            # small descriptor-generation cost
            scalar.wait_ge(vsem, 7)
            scalar.dma_start(out=out[1, :, 0:768],
                             in_=ob[1][:, 0:768]).then_inc(osem, 16)
            scalar.wait_ge(vsem, 8)
            scalar.dma_start(out=out[1, :, 768:1024],
                             in_=ob[1][:, 768:1024]).then_inc(osem, 16)
            # no final osem wait: the NRT postamble's per-engine DGE
            # drains guarantee the last write completes before NEFF end

        @block.tensor
        def _(tensor):
            # HAM warmup on garbage SBUF: PE must be busy for one FULL
            # free-running 3.4us activity window to clock up, and any idle
            # gap restarts the window — so bridge block entry -> first
            # real matmul (~3.8us) without pause.
            for i in range(N_WARMUP_MM):
                tensor.matmul(ps[7][0:64, 0:128], w_sb[:, 0:64],
                              w_sb[:, 256:384], start=True, stop=True,
                              tile_position=(0, 0))
            tensor.wait_ge(wsem_a, 16)
            for t, (b, p0, nph, ci) in enumerate(TILES):
                tensor.wait_ge(csem[ci], 16)
                if t == 0:
                    wb_wait = WA_STEPS  # wait wsem_b before this step
                else:
                    wb_wait = -1
                v = x2[b].rearrange("p (r f c) -> p r f c", f=4, c=33)
                # column-tiled pairs: step i runs pair i in PE columns
                # 0-63 and pair 9+i concurrently in columns 64-127
                for i in range(9):
                    if i == wb_wait:
                        tensor.wait_ge(wsem_b, 16)
                    for g in range(2):
                        j = 9 * g + i
                        a, sw = divmod(j, 6)
                        r0 = 2 * p0 + a
                        rhs = v[:, r0: r0 + 2 * nph - 1: 2, sw % 4,
                                sw // 4: sw // 4 + 32]
                        mm = tensor.matmul(
                            ps[t][64 * g:64 * g + 64, :],
                            w_sb[:, 128 * i + 64 * g:128 * i + 64 * g + 64],
                            rhs, start=(i == 0), stop=(i == 8),
                            tile_position=(0, 64 * g))
                        if i == 8:
                            # two incs per tile: DVE starts folding the
                            # finished half while the other drains
                            mm.then_inc(mmsem, 1)

        @block.vector
        def _(vector):
            vector.wait_ge(absem, 16)
            for t, (b, p0, nph, _) in enumerate(TILES):
                # group 0 (psum partitions 0-63) retires first: fold it
                # with the bias while group 1's last matmul drains.
                # DVE reads at most one PSUM operand per op.
                vector.wait_ge(mmsem, 2 * t + 1)
                vector.tensor_add(tmp[:], ps[t][0:64, :],
                                  ab_sb[:, p0 * 32:(p0 + nph) * 32])
                vector.wait_ge(mmsem, 2 * t + 2)
                vector.tensor_add(ob[b][:, p0 * 32:(p0 + nph) * 32],
                                  tmp[:], ps[t][64:128, :]).then_inc(vsem, 1)

    nc.compile()
    return nc


def _host_precompute(inputs):
    """Fold BN/alpha/bias into 6x6 stride-4 conv weights + bias map (f64)."""
    g0 = np.asarray(inputs["g0"], np.float64)
    b0 = np.asarray(inputs["b0"], np.float64)
    m0 = np.asarray(inputs["m0"], np.float64)
    v0 = np.asarray(inputs["v0"], np.float64)
    wv = np.asarray(inputs["wv"], np.float64)
    bv = np.asarray(inputs["bv"], np.float64)
    alpha = float(np.asarray(inputs["alpha"]))

    s0 = g0 / np.sqrt(v0 + EPS)
    t0 = b0 - m0 * s0

    # W'[o,c,sh,sw] = sum of 3x3 taps t with s - t in [0,4)^2
    Wp = np.zeros((C, C, 6, 6))
    for sh in range(6):
        for sw in range(6):
            th0, th1 = max(0, sh - 3), min(3, sh + 1)
            tw0, tw1 = max(0, sw - 3), min(3, sw + 1)
            Wp[:, :, sh, sw] = wv[:, :, th0:th1, tw0:tw1].sum(axis=(2, 3))

    W_final = (1.0 - alpha) * Wp * s0[None, :, None, None]
    idx = np.arange(C)
    for sh in range(1, 5):
        for sw in range(1, 5):
            W_final[idx, idx, sh, sw] += alpha

    # bias map: contribution of the BN shift t0 through the conv (with
    # zero-padding mask) plus conv bias, scaled by (1-alpha)
    Rm = np.zeros((OH, 6))
    for p in range(OH):
        for s in range(6):
            if 0 <= 4 * p + s - 1 < H:
                Rm[p, s] = 1.0
    A0 = np.einsum("ocuv,pu,qv,c->opq", Wp, Rm, Rm, t0)
    Abias = (1.0 - alpha) * (A0 + 16.0 * bv[:, None, None])

    # step-major lhsT layout: step i holds pair i (cols 0-63) then pair
    # 9+i (cols 64-127); pair j = (a, sw) = divmod(j, 6), rows 0-63 =
    # tap (2a, sw), rows 64-127 = tap (2a+1, sw); [k, m] = [ci, co]
    W18 = np.zeros((128, 18 * 64))
    for i in range(9):
        for g in range(2):
            j = 9 * g + i
            a, sw = divmod(j, 6)
            c0 = 128 * i + 64 * g
            W18[0:64, c0:c0 + 64] = W_final[:, :, 2 * a, sw].T
            W18[64:128, c0:c0 + 64] = W_final[:, :, 2 * a + 1, sw].T

    return W18, Abias.reshape(C, OH * OW)


def _fs_dither_e3m4(x):
    """Quantize x to fp8 e3m4 with 2D Floyd-Steinberg error diffusion.

    The output sums x over 4x4 patches (the alpha residual) and contracts
    it against slowly-varying 6x6 conv weights, so pushing the quantization
    error to high spatial frequency roughly halves the end-to-end error vs
    plain round-to-nearest."""
    import ml_dtypes

    v = np.asarray(x, np.float32).reshape(-1, H, W).copy()
    out = np.empty_like(v)
    sev = np.float32(7 / 16)
    dl, dc, dr = np.float32(3 / 16), np.float32(5 / 16), np.float32(1 / 16)
    for h in range(H):
        nxt = np.zeros((v.shape[0], W), np.float32)
        row = v[:, h]
        for w in range(W):
            val = row[:, w]
            qv = val.astype(ml_dtypes.float8_e3m4)
            out[:, h, w] = qv
            e = val - qv.astype(np.float32)
            if w + 1 < W:
                row[:, w + 1] += e * sev
                nxt[:, w + 1] += e * dr
            nxt[:, w] += e * dc
            if w > 0:
                nxt[:, w - 1] += e * dl
        if h + 1 < H:
            v[:, h + 1] += nxt
    return out.reshape(x.shape).astype(ml_dtypes.float8_e3m4)


def _host_shuffle_x(x):
    """Zero-padded h-parity, phase-major-column fp8 layout
    [B, 128*NROW*WPAD], chunk-major.

    Partition p < 64: channel p, even padded rows (pad row 2*r -> h=2r-1);
    partition p >= 64: channel p-64, odd padded rows (pad row 2*r+1 -> h=2r).
    Padded col c (data cols 1..128, zeros at 0/129/130/131) is stored at
    row offset (c%4)*33 + c//4 so stride-4 tap reads are contiguous.
    """
    import ml_dtypes

    xq = _fs_dither_e3m4(x)
    xpad = np.zeros((B, 128, NROW, WPAD), ml_dtypes.float8_e3m4)
    xpad[:, 0:64, 1:65, 1:129] = xq[:, :, 1::2, :]
    xpad[:, 64:128, 0:64, 1:129] = xq[:, :, 0::2, :]
    # c = cc*4 + phase -> phase-major [4][33]
    xph = np.ascontiguousarray(
        xpad.reshape(B, 128, NROW, 33, 4).transpose(0, 1, 2, 4, 3)
    ).reshape(B, 128, NROW, WPAD)
    # chunk-major: concatenate each row-chunk's [128, rows*WPAD] block so
    # the device reads one contiguous DRAM region per chunk DMA
    blocks = []
    for r0, r1 in HOST_CHUNKS:
        blocks.append(xph[:, :, r0:r1, :].reshape(B, 128 * (r1 - r0) * WPAD))
    return np.ascontiguousarray(np.concatenate(blocks, axis=1))


def _prepare_in_maps(inputs):
    x = np.asarray(inputs["x"], np.float32)
    W18, Abias = _host_precompute(inputs)
    w16 = W18.astype(np.float16)
    wacol = 128 * WA_STEPS
    wa_host = np.ascontiguousarray(w16[:, :wacol])
    wb_host = np.ascontiguousarray(w16[:, wacol:])
    ab_host = np.ascontiguousarray(Abias.astype(np.float16))
    xp = _host_shuffle_x(x)
    return [
        {"xp": xp[i * BPC:(i + 1) * BPC], "wa": wa_host, "wb": wb_host,
         "abias": ab_host}
        for i in range(NCORES)
    ]


def kernel(**inputs):
    from concourse.bass_utils import run_bass_kernel_spmd

    in_maps = _prepare_in_maps(inputs)

    if "nc" not in _PROGRAM_CACHE:
        _PROGRAM_CACHE["nc"] = _build_program()
    nc = _PROGRAM_CACHE["nc"]

    res = run_bass_kernel_spmd(nc, in_maps, list(range(NCORES)))
    out = np.concatenate(
        [np.asarray(res.results[i]["out"]).astype(np.float32).reshape(
            BPC, C, OH, OW) for i in range(NCORES)],
        axis=0,
    )
    return np.ascontiguousarray(out)
